# revision 24
# baseline (speedup 1.0000x reference)
"""GCN (4-layer, PyG-default GCNConv) forward on 8 Trainium2 NeuronCores.

Strategy (node-parallel, two-pass source-split pipeline):
  - Nodes are partitioned contiguously across the 8 cores (1250 rows each,
    padded to 1280 = 10 blocks of 128).
  - Per layer: each core computes its row-slice of G = H @ W as a tiled PE
    GEMM (bf16 in / fp32 accumulate). The slice is published in TWO
    AllGathers: rows 0:512 (blocks 0-3) fire mid-loop at block 3, rows
    512:1280 at block 9 (the 4/6 split fires the first collective earlier,
    trimming the next layer's wait for it). Both land in disjoint regions of
    ONE replicated [10240, w] HBM tensor (half-major layout).
  - Aggregation is split into two passes over the 10 destination blocks:
    pass A consumes only half-A source rows (in_ap = rows 0:4096, so the
    dependency is on the half-A collective alone and pass A overlaps the
    in-flight half-B collective), accumulating S_A.T @ MSG_A into PSUM and
    staging (+bias) into SBUF; pass B adds S_B.T @ MSG_B and fuses relu +
    the next-layer GEMM per block. This hides most of the ~60us
    fire-to-complete collective latency.
  - Gathers walk each pass's chunk sequence FLAT (calls may straddle block
    boundaries), 256 indices per dma_gather call (the HW sweet spot: bigger
    calls were measured slower, smaller ones pay the ~1us/call descriptor
    generation more often), round-robined over the 4 SWDGE queues. Source
    rows are deduplicated per (block, half) on the host (S carries edge
    multiplicity).
  - Layer 4 is reassociated as A_hat @ (H4 W4) so the final aggregation is
    128 wide (2 real classes padded); log_softmax is fused on-chip.
  - Constants (S, idx, W2-4, biases) load on the Scalar HWDGE queue at
    kernel start so they neither delay layer-1's x tiles (Sync queue) nor
    contend with the first collective.
"""

import sys

sys.path.insert(0, "/opt/trn_rl_repo")

import numpy as np
import ml_dtypes

BF16 = ml_dtypes.bfloat16
F8 = getattr(ml_dtypes, "float8_e4m3", None) or ml_dtypes.float8_e4m3fn

# Problem constants (nn_GCN_39195871543847)
N, E, F_IN, HID, C = 10000, 160000, 2208, 512, 2
W_CORES = 8
RPC = N // W_CORES  # 1250 nodes per core
MB = 10  # 128-row blocks per core
RPAD = MB * 128  # 1280
HB_A = 4  # blocks in half A (per core)
ROWS_A = HB_A * 128  # 512
ROWS_B = RPAD - ROWS_A  # 768
GLOB_A = W_CORES * ROWS_A  # 4096
GLOB_B = W_CORES * ROWS_B  # 6144
KFC = (F_IN + 127) // 128  # 18 contraction chunks for layer 1
KFP = KFC * 128  # 2304
C_PAD = 128  # pad 2 output classes to 128 bf16 (256B gather rows)
N_QUEUES = 4  # SWDGE queues for gather descriptor generation
GCALL = 2  # 128-idx chunks per dma_gather call (256-idx HW sweet spot)


def _install_drain_patch():
    """This container's walrus accepts at most one sync-wait per instruction;
    TileContext's final drain gets one wait per live semaphore. Split the
    extra waits onto single-wait NOPs."""
    import bass_rust
    import concourse.tile as tile
    from concourse.vector_clock import ScopedClock

    if getattr(tile.TileContext, "_drain_patch_installed", False):
        return

    def _drain_and_barrier(self, tick_clock, wait_clock):
        drain_inst = self.nc.sync.drain()
        wait_clock.add_sem_waits(
            drain_inst.ins, ScopedClock({None: tick_clock.global_clock})
        )
        si = drain_inst.ins.sync_info
        waits = list(si.on_wait or []) if si is not None else []
        if len(waits) > 1:
            si.on_wait = waits[:1]
            for w in waits[1:]:
                nop = self.nc.sync.nop(nofuse=True)
                nop.ins.sync_info = bass_rust.SyncInfo(on_wait=[w], on_update=[])
        self.nc.all_engine_barrier()
        assert self.sems is not None
        popped = self.nc._tile_sem_poison_stack.pop()
        assert popped is self._sem_poison
        self.nc.clear_and_free_semaphores(list(self.sems.allocated().values()))
        self.nc.all_engine_barrier()

    tile.TileContext._drain_and_barrier = _drain_and_barrier
    tile.TileContext._drain_patch_installed = True


# ----------------------------------------------------------------------------
# Host-side graph preprocessing
# ----------------------------------------------------------------------------


def _preprocess(edge_index):
    """Per core, per 128-dst block, per source half: dedup source rows and
    build the S chunk stack plus the dma_gather index layout.

    Chunk slots: half-A blocks 0..9 first (kaA[b] chunks each), then half-B
    blocks (kaB[b] chunks). Gather indices are half-local (A: 0..4095,
    B: 0..6143)."""
    src = edge_index[0].astype(np.int64)
    dst = edge_index[1].astype(np.int64)
    loop = np.arange(N, dtype=np.int64)
    s = np.concatenate([src, loop])
    d = np.concatenate([dst, loop])
    deg = np.bincount(d, minlength=N).astype(np.float32)
    dinv = np.where(deg > 0, 1.0 / np.sqrt(deg), 0.0).astype(np.float32)
    norm = (dinv[s] * dinv[d]).astype(np.float32)

    core = d // RPC
    per_core = []
    cnts = np.zeros((W_CORES, 3, MB), np.int64)
    for c in range(W_CORES):
        m = core == c
        sc, dc, wc = s[m], d[m] - c * RPC, norm[m]
        s_core = sc // RPC
        s_loc = sc % RPC
        local = s_core == c
        half = np.where(local, 0, (s_loc >= ROWS_A).astype(np.int64) + 1)
        # pass 0: own-core sources, indexed into own[l] (0..RPAD);
        # passes 1/2: remote sources, half-local indices into full[l]
        g_row = np.where(
            local,
            s_loc,
            np.where(
                s_loc < ROWS_A,
                s_core * ROWS_A + s_loc,
                s_core * ROWS_B + (s_loc - ROWS_A),
            ),
        )
        blk = dc // 128
        mloc = dc % 128
        ents = {}
        for h in (0, 1, 2):
            for b in range(MB):
                sel = (half == h) & (blk == b)
                uniq, inv = np.unique(g_row[sel], return_inverse=True)
                cnts[c, h, b] = max(1, len(uniq))
                ents[(h, b)] = (uniq, inv, mloc[sel], wc[sel])
        per_core.append(ents)

    ka0 = tuple(int(v) for v in (cnts[:, 0, :].max(0) + 127) // 128)
    kaA = tuple(int(v) for v in (cnts[:, 1, :].max(0) + 127) // 128)
    kaB = tuple(int(v) for v in (cnts[:, 2, :].max(0) + 127) // 128)
    T = sum(ka0) + sum(kaA) + sum(kaB)
    tb = {}
    off = 0
    for h, ka in ((0, ka0), (1, kaA), (2, kaB)):
        for b in range(MB):
            tb[(h, b)] = off
            off += ka[b]

    s_list, idx_list = [], []
    for c in range(W_CORES):
        S = np.zeros((T, 128, 128), np.float32)
        idx_flat = np.zeros(T * 128, np.int16)
        for (h, b), (uniq, inv, ms, ws) in per_core[c].items():
            t0 = tb[(h, b)]
            if len(uniq):
                kk = np.arange(len(uniq))
                idx_flat[t0 * 128 + kk] = uniq.astype(np.int16)
                np.add.at(S, (t0 + inv // 128, inv % 128, ms), ws)
        lay16 = idx_flat.reshape(T * 8, 16).T  # [16, T*8]
        idx_list.append(np.tile(lay16, (8, 1)).astype(np.int16))
        # SBUF-resident layout [128 partitions(k), T, 128(m)]
        s_list.append(np.ascontiguousarray(S.transpose(1, 0, 2)).astype(BF16))
    return ka0, kaA, kaB, s_list, idx_list


def _prep_inputs(x, edge_index, W1, b1, W2, b2, W3, b3, W4, b4):
    ka0, kaA, kaB, s_list, idx_list = _preprocess(edge_index)

    # xT per core: [MB, 128(p), KFC, 128(j)]; xT[m,p,k,j] = x[c*RPC+m*128+j, k*128+p]
    xts = []
    for c in range(W_CORES):
        xp = np.zeros((RPAD, KFP), np.float32)
        xp[:RPC, :F_IN] = x[c * RPC : (c + 1) * RPC]
        xt = xp.reshape(MB, 128, KFC, 128).transpose(0, 3, 2, 1)
        xts.append(np.ascontiguousarray(xt).astype(F8))

    W1p = np.zeros((KFP, HID), np.float32)
    W1p[:F_IN] = W1
    W1l = np.ascontiguousarray(
        W1p.reshape(KFC, 128, HID).transpose(1, 0, 2)
    ).astype(F8)
    W2l = np.ascontiguousarray(W2.reshape(4, 128, HID).transpose(1, 0, 2)).astype(BF16)
    W3l = np.ascontiguousarray(W3.reshape(4, 128, HID).transpose(1, 0, 2)).astype(BF16)
    W4p = np.zeros((HID, C_PAD), np.float32)
    W4p[:, :C] = W4
    W4l = np.ascontiguousarray(
        W4p.reshape(4, 128, C_PAD).transpose(1, 0, 2)
    ).astype(BF16)

    b1r = np.broadcast_to(b1, (128, HID)).astype(np.float32).copy()
    b2r = np.broadcast_to(b2, (128, HID)).astype(np.float32).copy()
    b3r = np.broadcast_to(b3, (128, HID)).astype(np.float32).copy()
    b4r = np.zeros((128, C_PAD), np.float32)
    b4r[:, :C] = b4

    in_maps = []
    for c in range(W_CORES):
        in_maps.append(
            {
                "xT": xts[c],
                "W1l": W1l, "W2l": W2l, "W3l": W3l, "W4l": W4l,
                "b1r": b1r, "b2r": b2r, "b3r": b3r, "b4r": b4r,
                "S_in": s_list[c],
                "idx_in": idx_list[c],
            }
        )
    return ka0, kaA, kaB, in_maps


# ----------------------------------------------------------------------------
# Bass kernel builder
# ----------------------------------------------------------------------------

_cache = {}


def _build(ka0, kaA, kaB):
    import concourse.bass as bass
    import concourse.mybir as mybir
    from concourse.bacc import Bacc
    from concourse.tile import TileContext
    from concourse.masks import make_identity

    f32 = mybir.dt.float32
    bf16 = mybir.dt.bfloat16
    f8 = mybir.dt.float8e4
    i16 = mybir.dt.int16

    T0, TA, TB = sum(ka0), sum(kaA), sum(kaB)
    T = T0 + TA + TB
    # per-pass flat chunk -> block maps and block first chunk slots
    blk0 = [b for b in range(MB) for _ in range(ka0[b])]
    blkA = [b for b in range(MB) for _ in range(kaA[b])]
    blkB = [b for b in range(MB) for _ in range(kaB[b])]
    tb0, tbA, tbB = [0] * MB, [0] * MB, [0] * MB
    for b in range(1, MB):
        tb0[b] = tb0[b - 1] + ka0[b - 1]
        tbA[b] = tbA[b - 1] + kaA[b - 1]
        tbB[b] = tbB[b - 1] + kaB[b - 1]

    nc = Bacc(num_devices=W_CORES, num_swdge_queues=N_QUEUES)
    gq = [0]  # round-robin cursor over gather queues

    xT = nc.dram_tensor("xT", [MB, 128, KFC, 128], f8, kind="ExternalInput")
    W1l = nc.dram_tensor("W1l", [128, KFC, HID], f8, kind="ExternalInput")
    W2l = nc.dram_tensor("W2l", [128, 4, HID], bf16, kind="ExternalInput")
    W3l = nc.dram_tensor("W3l", [128, 4, HID], bf16, kind="ExternalInput")
    W4l = nc.dram_tensor("W4l", [128, 4, C_PAD], bf16, kind="ExternalInput")
    b1r = nc.dram_tensor("b1r", [128, HID], f32, kind="ExternalInput")
    b2r = nc.dram_tensor("b2r", [128, HID], f32, kind="ExternalInput")
    b3r = nc.dram_tensor("b3r", [128, HID], f32, kind="ExternalInput")
    b4r = nc.dram_tensor("b4r", [128, C_PAD], f32, kind="ExternalInput")
    S_in = nc.dram_tensor("S_in", [128, T, 128], bf16, kind="ExternalInput")
    idx_in = nc.dram_tensor("idx_in", [128, T * 8], i16, kind="ExternalInput")
    out = nc.dram_tensor("out", [RPAD, C], f32, kind="ExternalOutput")

    # per-layer bounce buffers + replicated halves (layers 0..2 hold G, 3 G4)
    l_wid = [HID, HID, HID, C_PAD]
    # layers 0-2 bounce/replicate G in fp8e4m3 (halves gather + collective
    # bytes; final error stays ~1e-3, measured on host); layer 3 keeps bf16
    # for the 256B-row gather minimum.
    l_dt = [f8, f8, f8, bf16]
    own, full = [], []
    for l in range(4):
        own.append(
            nc.dram_tensor(f"own{l}", [RPAD, l_wid[l]], l_dt[l], kind="Internal")
        )
        full.append(
            nc.dram_tensor(
                f"full{l}", [GLOB_A + GLOB_B, l_wid[l]], l_dt[l], kind="Internal",
                addr_space="Shared",
            )
        )

    warm_own = nc.dram_tensor("warm_own", [8, 64], bf16, kind="Internal")
    warm_full = nc.dram_tensor(
        "warm_full", [64, 64], bf16, kind="Internal", addr_space="Shared"
    )

    rg = [list(range(W_CORES))]

    with TileContext(nc) as tc:
        with (
            tc.tile_pool(name="const", bufs=1) as cpool,
            tc.tile_pool(name="work", bufs=2) as wpool,
            tc.tile_pool(name="psum", bufs=2, space="PSUM") as ppool,
        ):
            relu = mybir.ActivationFunctionType.Relu

            regs = {n: nc.gpsimd.to_reg(n * 128) for n in (1, 2)}

            # fire-and-forget warmup collective: pays the first-collective
            # setup latency during the layer-1 GEMM instead of after it
            nc.gpsimd.collective_compute(
                "AllGather",
                mybir.AluOpType.bypass,
                ins=[warm_own[:]],
                outs=[warm_full[:]],
                replica_groups=rg,
            )

            # ---- resident tensors: W1 + x tiles gate the PE, so they go on
            # the Sync queue; everything else streams on the Scalar HWDGE
            # queue during the layer-1 GEMM --------------------------------
            W1_sb = cpool.tile([128, KFC, HID], f8)
            nc.sync.dma_start(out=W1_sb[:, :4, :], in_=W1l[:, :4, :])
            nc.sync.dma_start(out=W1_sb[:, 4:, :], in_=W1l[:, 4:, :])
            S_sb = cpool.tile([128, T, 128], bf16)
            nc.scalar.dma_start(out=S_sb[:], in_=S_in[:])
            idx_sb = cpool.tile([128, T * 8], i16)
            nc.scalar.dma_start(out=idx_sb[:], in_=idx_in[:])
            W2_sb = cpool.tile([128, 4, HID], bf16)
            nc.scalar.dma_start(out=W2_sb[:], in_=W2l[:])
            W3_sb = cpool.tile([128, 4, HID], bf16)
            nc.scalar.dma_start(out=W3_sb[:], in_=W3l[:])
            W4_sb = cpool.tile([128, 4, C_PAD], bf16)
            nc.scalar.dma_start(out=W4_sb[:], in_=W4l[:])
            b_sb = []
            for nm, srcb in (("b1", b1r), ("b2", b2r), ("b3", b3r)):
                t = cpool.tile([128, HID], f32, tag=f"bias_{nm}")
                nc.scalar.dma_start(out=t[:], in_=srcb[:])
                b_sb.append(t)
            b4_sb = cpool.tile([128, C_PAD], f32)
            nc.scalar.dma_start(out=b4_sb[:], in_=b4r[:])
            id_bf = cpool.tile([128, 128], bf16)
            make_identity(nc, id_bf[:])
            # SBUF staging for pass-A partial aggregates (psumA + bias)
            stage = cpool.tile([128, MB, HID], f32)
            stage4 = cpool.tile([128, MB, C_PAD], f32)

            def allgather_half(l, h):
                if h == 0:
                    ins_ap = own[l][0:ROWS_A, :]
                    outs_ap = full[l][0:GLOB_A, :]
                else:
                    ins_ap = own[l][ROWS_A:RPAD, :]
                    outs_ap = full[l][GLOB_A : GLOB_A + GLOB_B, :]
                nc.gpsimd.collective_compute(
                    "AllGather",
                    mybir.AluOpType.bypass,
                    ins=[ins_ap],
                    outs=[outs_ap],
                    replica_groups=rg,
                )

            def store_own(lslot, b, tile):
                nc.sync.dma_start(
                    out=own[lslot][b * 128 : (b + 1) * 128, :], in_=tile[:]
                )
                if b == HB_A - 1:
                    allgather_half(lslot, 0)
                elif b == MB - 1:
                    allgather_half(lslot, 1)

            def gemm_l1():
                for m in range(MB):
                    xm = wpool.tile([128, KFC, 128], f8, tag="xm", bufs=4)
                    nc.sync.dma_start(out=xm[:], in_=xT[m])
                    ps = ppool.tile([128, HID], f32, tag="gps")
                    # fp8 DoubleRow: two contraction chunks per matmul at 2x rate
                    for k2 in range(KFC // 2):
                        nc.tensor.matmul(
                            ps[:],
                            lhsT=xm[:, 2 * k2 : 2 * k2 + 2, :],
                            rhs=W1_sb[:, 2 * k2 : 2 * k2 + 2, :],
                            start=(k2 == 0),
                            stop=(k2 == KFC // 2 - 1),
                            perf_mode=mybir.MatmulPerfMode.DoubleRow,
                        )
                    gb = wpool.tile([128, HID], f8, tag="gb", bufs=3)
                    nc.scalar.copy(gb[:], ps[:])
                    store_own(0, m, gb)

            gemm_l1()

            def logsoftmax_block(ps_ap, stage_sl, m):
                lg = wpool.tile([128, C_PAD], f32, tag="lg")
                nc.vector.tensor_add(out=lg[:], in0=ps_ap, in1=stage_sl)
                mx = wpool.tile([128, 1], f32, tag="mx")
                nc.vector.tensor_reduce(
                    out=mx[:], in_=lg[:, :C], axis=mybir.AxisListType.X,
                    op=mybir.AluOpType.max,
                )
                t2 = wpool.tile([128, C], f32, tag="t2")
                nc.vector.tensor_scalar(
                    out=t2[:], in0=lg[:, :C], scalar1=mx[:], scalar2=None,
                    op0=mybir.AluOpType.subtract,
                )
                e2 = wpool.tile([128, C], f32, tag="e2")
                nc.scalar.activation(e2[:], t2[:], mybir.ActivationFunctionType.Exp)
                sm = wpool.tile([128, 1], f32, tag="sm")
                nc.vector.tensor_reduce(
                    out=sm[:], in_=e2[:], axis=mybir.AxisListType.X,
                    op=mybir.AluOpType.add,
                )
                ls = wpool.tile([128, 1], f32, tag="ls")
                nc.scalar.activation(ls[:], sm[:], mybir.ActivationFunctionType.Ln)
                o2 = wpool.tile([128, C], f32, tag="o2")
                nc.vector.tensor_scalar(
                    out=o2[:], in0=t2[:], scalar1=ls[:], scalar2=None,
                    op0=mybir.AluOpType.subtract,
                )
                nc.sync.dma_start(out=out[m * 128 : (m + 1) * 128, :], in_=o2[:])

            def flat_pass(src_ap, base, tb, ka, blk, Ttot, w, dt, finish_block):
                """Walk the pass's Ttot chunks flat: one gather per GCALL
                chunks (calls straddle block boundaries), matmuls routed to
                each chunk's block psum; finish_block fires at each block's
                last chunk."""
                ps_by_block = {}
                t0 = 0
                while t0 < Ttot:
                    ngc = min(GCALL, Ttot - t0)
                    tg = base + t0
                    msg = wpool.tile(
                        [128, GCALL, w], dt, tag=f"msg{w}", bufs=28 // GCALL
                    )
                    nc.gpsimd.dma_gather(
                        out_ap=msg[:, :ngc, :],
                        in_ap=src_ap,
                        idxs_ap=idx_sb[:, tg * 8 : (tg + ngc) * 8],
                        num_idxs=ngc * 128,
                        num_idxs_reg=regs[ngc],
                        elem_size=w,
                        queue_num=gq[0],
                    )
                    gq[0] = (gq[0] + 1) % N_QUEUES
                    for u in range(ngc):
                        t = t0 + u
                        b = blk[t]
                        if t == tb[b]:
                            ps_by_block[b] = ppool.tile(
                                [128, HID], f32, name="aps", tag="aps", bufs=5
                            )
                        nc.tensor.matmul(
                            ps_by_block[b][:, :w],
                            lhsT=S_sb[:, base + t, :],
                            rhs=msg[:, u, :],
                            start=(t == tb[b]),
                            stop=(t == tb[b] + ka[b] - 1),
                        )
                        if t == tb[b] + ka[b] - 1:
                            finish_block(b, ps_by_block.pop(b))
                    t0 += ngc

            def layer(l, bias_t, mode):
                """Two-pass fused layer. Pass A: aggregate half-A sources into
                PSUM, stage (+bias) to SBUF — depends only on the half-A
                collective, so it overlaps the in-flight half-B collective.
                Pass B: aggregate half-B, combine with the staged partials,
                then relu + next-layer GEMM (or final log_softmax) per block;
                the next layer's half collectives fire at blocks 3 and 9.
                mode: ("gemm", w_sb, lnext) | ("final",)"""
                final = mode[0] == "final"
                w = C_PAD if final else HID
                st = stage4 if final else stage
                srcA = full[l][0:GLOB_A, :]
                srcB = full[l][GLOB_A : GLOB_A + GLOB_B, :]

                def finish_0(b, ps):
                    nc.vector.tensor_add(
                        out=st[:, b, :], in0=ps[:, :w], in1=bias_t[:]
                    )

                def finish_a(b, ps):
                    nc.vector.tensor_add(
                        out=st[:, b, :], in0=ps[:, :w], in1=st[:, b, :]
                    )

                def finish_b(b, ps):
                    if final:
                        logsoftmax_block(ps[:, :w], st[:, b, :], b)
                        return
                    hf = wpool.tile([128, HID], f32, tag="hf", bufs=3)
                    nc.vector.tensor_add(out=hf[:], in0=ps[:], in1=st[:, b, :])
                    hb = wpool.tile([128, HID], bf16, tag="hb", bufs=3)
                    nc.scalar.activation(hb[:], hf[:], relu)
                    # transpose into GEMM lhsT layout
                    ht = wpool.tile([128, 4, 128], bf16, tag="ht", bufs=4)
                    for g in range(4):
                        tp = ppool.tile([128, 128], bf16, tag="tps", bufs=1)
                        nc.tensor.transpose(
                            tp[:], hb[:, g * 128 : (g + 1) * 128], id_bf[:]
                        )
                        nc.vector.tensor_copy(out=ht[:, g, :], in_=tp[:])
                    _, w_sb, lnext = mode
                    wid = l_wid[lnext]
                    gp = ppool.tile([128, HID], f32, tag="gps")
                    for k in range(4):
                        nc.tensor.matmul(
                            gp[:, :wid],
                            lhsT=ht[:, k, :],
                            rhs=w_sb[:, k, :],
                            start=(k == 0),
                            stop=(k == 3),
                        )
                    gb = wpool.tile([128, wid], l_dt[lnext], tag="gb2", bufs=3)
                    nc.scalar.copy(gb[:], gp[:, :wid])
                    store_own(lnext, b, gb)

                dt = l_dt[l]
                flat_pass(own[l][:], 0, tb0, ka0, blk0, T0, w, dt, finish_0)
                flat_pass(srcA, T0, tbA, kaA, blkA, TA, w, dt, finish_a)
                flat_pass(srcB, T0 + TA, tbB, kaB, blkB, TB, w, dt, finish_b)

            # ---- layers ----------------------------------------------------
            layer(0, b_sb[0], ("gemm", W2_sb, 1))
            layer(1, b_sb[1], ("gemm", W3_sb, 2))
            layer(2, b_sb[2], ("gemm", W4_sb, 3))
            layer(3, b4_sb, ("final",))

    nc.compile()
    return nc


# ----------------------------------------------------------------------------
# Entry point
# ----------------------------------------------------------------------------


def kernel(x, edge_index, batch, W1, b1, W2, b2, W3, b3, W4, b4, _trace=False):
    _install_drain_patch()
    from concourse.bass_utils import run_bass_kernel_spmd

    ka0, kaA, kaB, in_maps = _prep_inputs(
        np.asarray(x, np.float32),
        np.asarray(edge_index),
        np.asarray(W1, np.float32), np.asarray(b1, np.float32),
        np.asarray(W2, np.float32), np.asarray(b2, np.float32),
        np.asarray(W3, np.float32), np.asarray(b3, np.float32),
        np.asarray(W4, np.float32), np.asarray(b4, np.float32),
    )
    key = (ka0, kaA, kaB)
    if key not in _cache:
        _cache[key] = _build(ka0, kaA, kaB)
    nc = _cache[key]
    res = run_bass_kernel_spmd(
        nc, in_maps, core_ids=list(range(W_CORES)), trace=_trace
    )
    outp = np.concatenate(
        [res.results[c]["out"][:RPC] for c in range(W_CORES)], axis=0
    ).astype(np.float32)
    if _trace:
        return outp, res
    return outp
